# revision 8
# baseline (speedup 1.0000x reference)
"""Trainium2 Bass kernel for nn_AdaptedLinear (hypernetwork-adapted linear).

Math (per sample b):
  h = emb_id[HN_ids[b]] + emb_layer[layer_id]                 # [256]
  A = (h @ W_A).reshape(R, IN)    t = A @ x_b                 # [16]
  B = (h @ W_B).reshape(OUT, R)
  out_b = weight @ x_b + B @ t + bias                         # never materialize delta

Distribution across 8 NeuronCores -- no collectives:
  - LoRA path sharded by rank: core c owns ranks {2c, 2c+1}; each core
    emits a partial lora [batch, out_full] with its base shard added at
    cols [0:256) of a per-core-rotated layout (wb columns rolled by
    -c*OSH on host); host un-rotates, sums, and adds bias.
  - weight (base path) sharded by output dim (256 cols/core).

v5 schedule:
  - Bulk streams on the two HWDGE rings (sync=SP, scalar=ACT), draining
    concurrently at ~360-400 GB/s aggregate.
    ring1 (sync):  [s1|wa ics0-7], [wa ics8-15], wb colsA (banks 0-1),
                   wb colB1 (bank 2), wb colB2 (bank 3), out x3
    ring2 (scalar): [s2|ht|h|dmask], wt
  - Q accumulates in two psum groups (ics 0-7 / 8-15) so half of the
    t-reduction runs during the second wa chunk's DMA wait.
  - base matmuls (own psum bank) slot between Q and the lora matmuls,
    overlapping the t->trep->g DVE chain.
  - every psum surface is its own 1-bank tile so PE writes to bank N
    never serialize against DVE reads of bank N-1.
  - fp8 matmuls (Q, lora) use DoubleRow; 11 dummy matmuls at t=0 warm
    the PE HAM clock gate and bridge to the first wa completion sem.
  - output bf16, 3 DMAs: cols [512:1536] early, [0:512], [1536:2048].
"""

import sys

sys.path.insert(0, "/opt/trn_rl_repo")

import numpy as np

import concourse.bass as bass
import concourse.bacc as bacc
import concourse.tile as tile
import concourse.mybir as mybir
from concourse.bass_utils import run_bass_kernel_spmd

IN_F, OUT_F, R = 2048, 2048, 16
HDIM = 256
BATCH = 16
N_CORES = 8
OSH = OUT_F // N_CORES     # 256 base-output cols per core
RL = R // N_CORES          # 2 local ranks per core
KL = RL * HDIM             # 512 local lora contraction rows

DT_W = mybir.dt.bfloat16
DT_F8 = mybir.dt.float8e4
WB_SCALE = 256.0
G_SCALE = 64.0
WA_SCALE = 256.0
LORA_DESCALE = 1.0 / (WB_SCALE * G_SCALE)
BASE_SCALE = WB_SCALE * G_SCALE
DT_WT = mybir.dt.bfloat16

IC = IN_F // 128           # 16 i-chunks
N_DUMMY = 8               # PE warm-up matmuls (N=512 each)
DR = mybir.MatmulPerfMode.DoubleRow

S1C = IC * BATCH           # 256 cols of s1 inside the wa tensor
S2X = IC * BATCH           # x cols in s2
S2H = 2 * BATCH            # ht cols in s2
S2HD = HDIM + BATCH * RL   # h + dmask cols (rows 0:16 only)


def _np_dt(dt):
    return np.dtype(mybir.dt.np(dt))


def _build():
    nc = bacc.Bacc("TRN2", target_bir_lowering=False, debug=False,
                   num_devices=N_CORES)
    f32 = mybir.dt.float32

    wa = nc.dram_tensor("wa", [128, S1C + IC * KL], DT_F8,
                        kind="ExternalInput")
    wb = nc.dram_tensor("wb", [128, 4 * OUT_F], DT_F8, kind="ExternalInput")
    s2 = nc.dram_tensor("s2", [128, S2X + S2H + S2HD], DT_WT,
                        kind="ExternalInput")
    wt = nc.dram_tensor("wt", [128, IC * OSH], DT_WT, kind="ExternalInput")
    out = nc.dram_tensor("out", [BATCH, OUT_F], DT_W, kind="ExternalOutput")

    with tile.TileContext(nc) as tc:
        with (
            tc.tile_pool(name="sb", bufs=1) as sb,
            tc.tile_pool(name="ps", bufs=1, space="PSUM") as ps,
        ):
            # psum: 8 banks, one tile each (dummies share trep's bank)
            qa_ps = ps.tile([BATCH, 512], f32, name="qa", tag="qa")
            qb_ps = ps.tile([BATCH, 512], f32, name="qb", tag="qb")
            trep_ps = ps.tile([128, 512], f32, name="trep", tag="trep")
            lora_ps = [ps.tile([BATCH, 512], f32, name=f"lo{n}", tag=f"lo{n}")
                       for n in range(4)]

            # ---- PE warm-up: dummy matmuls on memset junk (no DMA deps).
            junk_w = sb.tile([128, BATCH], DT_F8)
            nc.vector.memset(junk_w[:], 0.0)
            junk_r = sb.tile([128, 512], DT_F8)
            nc.gpsimd.memset(junk_r[:], 0.0)
            for i in range(N_DUMMY):
                nc.tensor.matmul(trep_ps[:BATCH, :], junk_w[:], junk_r[:],
                                 start=True, stop=True)
            # pre-clear lora bank0 cols [256:512): base clears [0:256)
            # itself; lora bank0 later accumulates with start=False.
            nc.tensor.matmul(lora_ps[0][:, OSH:], junk_w[:, :BATCH],
                             junk_r[:, :OSH], start=True, stop=True)

            # ---- DMA ring1 (sync): s1+wa, wb.  ring2 (scalar): s2, wt.
            wa_sb = sb.tile([128, S1C + IC * KL], DT_F8)
            half = S1C + 8 * KL
            q34 = S1C + 12 * KL
            nc.sync.dma_start(wa_sb[:, :half], wa[:, :half])
            nc.sync.dma_start(wa_sb[:, half:q34], wa[:, half:q34])
            nc.sync.dma_start(wa_sb[:, q34:], wa[:, q34:])
            wb_sb = sb.tile([128, 4 * OUT_F], DT_F8)
            nc.sync.dma_start(wb_sb[:, :2 * 2048], wb[:, :2 * 2048])
            nc.sync.dma_start(wb_sb[:, 2 * 2048:3 * 2048],
                              wb[:, 2 * 2048:3 * 2048])
            nc.sync.dma_start(wb_sb[:, 3 * 2048:], wb[:, 3 * 2048:])

            s2_sb = sb.tile([128, S2X + S2H + S2HD], DT_WT)
            nc.scalar.dma_start(s2_sb[:], s2[:])
            wt_sb = sb.tile([128, IC * OSH], DT_WT)
            nc.scalar.dma_start(wt_sb[:, :8 * OSH], wt[:, :8 * OSH])
            nc.scalar.dma_start(wt_sb[:, 8 * OSH:], wt[:, 8 * OSH:])

            # ---- Q phase (DoubleRow), two psum groups so t_a overlaps
            # the second wa chunk's DMA.
            s1_ap = wa_sb[:, :S1C]

            def q_mm(ic2, dst, start, stop):
                nc.tensor.matmul(
                    dst,
                    s1_ap[:, ic2 * 32:(ic2 + 1) * 32]
                    .rearrange("p (k b) -> p k b", k=2),
                    wa_sb[:, S1C + ic2 * 2 * KL:S1C + (ic2 + 1) * 2 * KL]
                    .rearrange("p (k n) -> p k n", k=2),
                    start=start, stop=stop, perf_mode=DR)

            for ic2 in range(4):
                q_mm(ic2, qa_ps[:], ic2 == 0, ic2 == 3)
            for ic2 in range(4, 8):
                q_mm(ic2, qb_ps[:], ic2 == 4, ic2 == 7)

            # ---- base phase, first half (wt ics 0-7): fills the PE idle
            # window while the t-chain runs on DVE.
            def base_mm(ic):
                nc.tensor.matmul(
                    lora_ps[0][:, :OSH],
                    s2_sb[:, ic * BATCH:(ic + 1) * BATCH],
                    wt_sb[:, ic * OSH:(ic + 1) * OSH],
                    start=(ic == 0), stop=(ic == IC - 1),
                    skip_group_check=True,
                )
            for ic in range(8):
                base_mm(ic)

            # ---- t[b,r] = sum_d (qa+qb)[b,(r,d)] * h[b,d]   (DVE)
            h_ap = s2_sb[:BATCH, S2X + S2H:S2X + S2H + HDIM]
            dm_ap = s2_sb[:BATCH, S2X + S2H + HDIM:]
            ta_sb = sb.tile([BATCH, RL], f32)
            tb_sb = sb.tile([BATCH, RL], f32)
            t_sb = sb.tile([BATCH, RL], f32)
            tt_scr = sb.tile([BATCH, HDIM], f32)
            for r in range(RL):
                nc.vector.scalar_tensor_tensor(
                    out=tt_scr[:], in0=qa_ps[:, r * HDIM:(r + 1) * HDIM],
                    scalar=1.0, in1=h_ap,
                    op0=mybir.AluOpType.mult, op1=mybir.AluOpType.mult,
                    accum_out=ta_sb[:, r:r + 1])
            for r in range(RL):
                nc.vector.scalar_tensor_tensor(
                    out=tt_scr[:], in0=qb_ps[:, r * HDIM:(r + 1) * HDIM],
                    scalar=1.0, in1=h_ap,
                    op0=mybir.AluOpType.mult, op1=mybir.AluOpType.mult,
                    accum_out=tb_sb[:, r:r + 1])
            nc.vector.tensor_add(t_sb[:], ta_sb[:], tb_sb[:])

            # ---- replicate t across partitions: trep = ones16.T @ (dm * t)
            ones16 = sb.tile([BATCH, 128], DT_W)
            nc.vector.memset(ones16[:], 1.0)
            rhs_t = sb.tile([BATCH, BATCH * RL], DT_W)
            nc.vector.tensor_mul(
                rhs_t[:].rearrange("k (b r) -> k b r", r=RL),
                dm_ap.rearrange("k (b r) -> k b r", r=RL),
                t_sb[:].unsqueeze(1).broadcast_to((BATCH, BATCH, RL)))
            nc.tensor.matmul(trep_ps[:, :BATCH * RL], ones16[:], rhs_t[:],
                             start=True, stop=True)
            # gT[(dh,p), (r, dh', b)] = h[b, dh'*128+p] * t[b, r]
            ht_ap = s2_sb[:, S2X:S2X + S2H]
            g_sb = sb.tile([128, RL * 2 * BATCH], DT_F8)
            nc.vector.tensor_mul(
                g_sb[:].rearrange("p (r k b) -> p r k b", r=RL, k=2),
                ht_ap.rearrange("p (k b) -> p k b", k=2)
                .unsqueeze(1).broadcast_to((128, RL, 2, BATCH)),
                trep_ps[:, :BATCH * RL].rearrange("p (b r) -> p r b", r=RL)
                .unsqueeze(2).broadcast_to((128, RL, 2, BATCH)))

            # ---- base second half (wt ics 8-15)
            for ic in range(8, IC):
                base_mm(ic)

            # ---- lora phase (DoubleRow), banks 1..3 then bank0 (bank0
            # accumulates onto base, start=False).  wb layout is
            # column-major: wb_sb[:, nn*2048 + kc*512 : +512].
            def lora_mms(nn):
                wb_v = wb_sb[:, nn * 2048:(nn + 1) * 2048] \
                    .rearrange("p (kc n) -> p kc n", kc=4)
                for kcc in range(2):
                    nc.tensor.matmul(
                        lora_ps[nn][:],
                        g_sb[:, kcc * 32:(kcc + 1) * 32]
                        .rearrange("p (k b) -> p k b", k=2),
                        wb_v[:, 2 * kcc:2 * kcc + 2, :],
                        start=(kcc == 0 and nn != 0), stop=(kcc == 1),
                        perf_mode=DR, skip_group_check=True)

            out_sb = sb.tile([BATCH, OUT_F], DT_W)
            for nn in (1, 2, 3):
                lora_mms(nn)
            lora_mms(0)

            nc.vector.tensor_scalar_mul(
                out_sb[:, 512:1024], lora_ps[1][:], LORA_DESCALE)
            nc.vector.tensor_scalar_mul(
                out_sb[:, 1024:1536], lora_ps[2][:], LORA_DESCALE)
            nc.sync.dma_start(out[:, 512:1536], out_sb[:, 512:1536])
            nc.vector.tensor_scalar_mul(
                out_sb[:, 1536:], lora_ps[3][:], LORA_DESCALE)
            nc.sync.dma_start(out[:, 1536:], out_sb[:, 1536:])
            nc.vector.tensor_scalar_mul(
                out_sb[:, :512], lora_ps[0][:], LORA_DESCALE)
            nc.sync.dma_start(out[:, :512], out_sb[:, :512])

    nc.compile()
    return nc


_NC_CACHE = None


def _get_nc():
    global _NC_CACHE
    if _NC_CACHE is None:
        _NC_CACHE = _build()
    return _NC_CACHE


def _interleave(a, p=128):
    """[C*p, F] -> [p, C*F]: the SBUF layout used on device."""
    c = a.shape[0] // p
    return np.ascontiguousarray(
        a.reshape(c, p, a.shape[1]).transpose(1, 0, 2).reshape(p, -1))


def _prep(x, HN_ids, layer_id, weight, bias, emb_id, emb_layer, W_A, W_B):
    """Host-side layout prep + sharding. Returns in_maps for 8 cores."""
    f32 = np.float32
    x = np.asarray(x, f32)
    weight = np.asarray(weight, f32)
    emb_id = np.asarray(emb_id, f32)
    emb_layer = np.asarray(emb_layer, f32)
    W_A = np.asarray(W_A, f32)
    W_B = np.asarray(W_B, f32)
    ids = np.asarray(HN_ids).astype(np.int64)
    lid = int(np.asarray(layer_id))

    h = emb_id[ids] + emb_layer[lid]                      # [B, HDIM]

    np_wt, np_f8 = _np_dt(DT_WT), _np_dt(DT_F8)

    s1 = _interleave(np.ascontiguousarray(x.T)).astype(np_f8)
    wa3 = W_A.reshape(HDIM, R, IN_F)
    wa_all = np.ascontiguousarray(
        wa3.transpose(2, 1, 0) * WA_SCALE).astype(np_f8)
    wb3 = W_B.reshape(HDIM, OUT_F, R)
    wb_all = np.ascontiguousarray(
        wb3.transpose(2, 0, 1) * WB_SCALE).astype(np_f8)
    ht = _interleave(np.ascontiguousarray(h.T)).astype(np_wt)
    xt_il = _interleave(np.ascontiguousarray(x.T)).astype(np_wt)
    dm = np.zeros((BATCH, BATCH, RL), f32)
    dm[np.arange(BATCH), np.arange(BATCH), :] = G_SCALE
    hd = np.zeros((128, S2HD), f32)
    hd[:BATCH, :HDIM] = h / WA_SCALE
    hd[:BATCH, HDIM:] = dm.reshape(BATCH, BATCH * RL)
    s2 = np.concatenate([xt_il, ht, hd.astype(np_wt)], axis=1)
    wt_full = np.ascontiguousarray(weight.T) * BASE_SCALE  # [in, out]

    in_maps = []
    for c in range(N_CORES):
        sl = slice(c * OSH, (c + 1) * OSH)
        rsl = slice(c * RL, (c + 1) * RL)
        wb_c = np.roll(wb_all[rsl], -c * OSH, axis=2).reshape(KL, OUT_F)
        wb_il = np.concatenate(
            [wb_c[kc * 128:(kc + 1) * 128, nn * 512:(nn + 1) * 512]
             for nn in range(4) for kc in range(4)], axis=1)
        wa_c = _interleave(np.ascontiguousarray(
            wa_all[:, rsl, :]).reshape(IN_F, KL))
        in_maps.append({
            "wa": np.concatenate([s1, wa_c], axis=1),
            "wb": np.ascontiguousarray(wb_il),
            "s2": s2,
            "wt": _interleave(
                np.ascontiguousarray(wt_full[:, sl]).astype(np_wt)),
        })
    return in_maps


def kernel(**inputs):
    nc = _get_nc()
    in_maps = _prep(**inputs)
    res = run_bass_kernel_spmd(nc, in_maps, core_ids=list(range(N_CORES)))
    bias = np.asarray(inputs["bias"], np.float32)
    out = np.zeros((BATCH, OUT_F), np.float32)
    for c in range(N_CORES):
        out += np.roll(np.asarray(res.results[c]["out"], np.float32),
                       c * OSH, axis=1)
    return (out + bias[None, :]).astype(np.float32)


def run_traced(inputs, n=3):
    """Timing helper for test.py: returns (exec_times_ns, last_results)."""
    nc = _get_nc()
    in_maps = _prep(**inputs)
    times = []
    res = None
    for _ in range(n):
        res = run_bass_kernel_spmd(nc, in_maps, core_ids=list(range(N_CORES)),
                                   trace=True)
        times.append(res.exec_time_ns)
    return times, res


# revision 9
# speedup vs baseline: 1.0003x; 1.0003x over previous
"""Trainium2 Bass kernel for nn_AdaptedLinear (hypernetwork-adapted linear).

Math (per sample b):
  h = emb_id[HN_ids[b]] + emb_layer[layer_id]                 # [256]
  A = (h @ W_A).reshape(R, IN)    t = A @ x_b                 # [16]
  B = (h @ W_B).reshape(OUT, R)
  out_b = weight @ x_b + B @ t + bias                         # never materialize delta

Distribution across 8 NeuronCores -- no collectives:
  - LoRA path sharded by rank: core c owns ranks {2c, 2c+1}; each core
    emits a partial lora [batch, out_full] in a per-core-rotated layout
    (wb columns rolled by -c*OSH on host); the base shard ships as a
    separate small output.  Host un-rotates, sums, adds base and bias.
  - weight (base path) sharded by output dim (256 cols/core).

v7 schedule:
  - Bulk streams on the two HWDGE rings (sync=SP, scalar=ACT), <=5
    dispatches per ring (deeper queues stall the dispatcher for ~3-5us).
    ring1 (sync):  [s1|wa ics0-7], [wa ics8-15], wb, out_main, out_base
    ring2 (scalar): [s2|ht|h|dmask], wt ics0-7, wt ics8-15
  - Q (DoubleRow fp8) accumulates in two psum groups; the first half of
    the t-reduction runs during the second wa chunk's DMA wait.
  - lora: 16 plain fp8 matmuls col-GROUP-TILED across the PE array --
    bank nn writes lora_all[32nn:32nn+16, :512] with
    tile_position=(0,32nn), so the four banks execute concurrently in
    the four 32-column sub-array groups of ONE psum bank.  The whole
    lora tail after the wb sem is ~1.2us.
  - result copy is ONE [128,512] DVE op (full 128 partitions) and the
    main output DMA is [128,512] bf16 = 128KB at full partition rate.
  - base matmuls (own bank) fill PE idle windows (wt in 2 chunks);
    base ships as its own [16,256] bf16 output, added on host.
  - 8 dummy matmuls at t=0 warm the PE HAM clock gate.
"""

import sys

sys.path.insert(0, "/opt/trn_rl_repo")

import numpy as np

import concourse.bass as bass
import concourse.bacc as bacc
import concourse.tile as tile
import concourse.mybir as mybir
from concourse.bass_utils import run_bass_kernel_spmd

IN_F, OUT_F, R = 2048, 2048, 16
HDIM = 256
BATCH = 16
N_CORES = 8
OSH = OUT_F // N_CORES     # 256 base-output cols per core
RL = R // N_CORES          # 2 local ranks per core
KL = RL * HDIM             # 512 local lora contraction rows

DT_W = mybir.dt.bfloat16
DT_F8 = mybir.dt.float8e4
WB_SCALE = 256.0
G_SCALE = 64.0
WA_SCALE = 256.0
LORA_DESCALE = 1.0 / (WB_SCALE * G_SCALE)
DT_WT = mybir.dt.bfloat16

IC = IN_F // 128           # 16 i-chunks
N_DUMMY = 8                # PE warm-up matmuls (N=512 each)
DR = mybir.MatmulPerfMode.DoubleRow

S1C = IC * BATCH           # 256 cols of s1 inside the wa tensor
S2X = IC * BATCH           # x cols in s2
S2H = 2 * BATCH            # ht cols in s2
S2HD = HDIM + BATCH * RL   # h + dmask cols (rows 0:16 only)


def _np_dt(dt):
    return np.dtype(mybir.dt.np(dt))


def _build():
    nc = bacc.Bacc("TRN2", target_bir_lowering=False, debug=False,
                   num_devices=N_CORES)
    f32 = mybir.dt.float32

    wa = nc.dram_tensor("wa", [128, S1C + IC * KL], DT_F8,
                        kind="ExternalInput")
    wb = nc.dram_tensor("wb", [128, 4 * OUT_F], DT_F8, kind="ExternalInput")
    s2 = nc.dram_tensor("s2", [128, S2X + S2H + S2HD], DT_WT,
                        kind="ExternalInput")
    wt = nc.dram_tensor("wt", [128, IC * OSH], DT_WT, kind="ExternalInput")
    out_m = nc.dram_tensor("out_m", [128, 512], DT_W, kind="ExternalOutput")
    out_b = nc.dram_tensor("out_b", [BATCH, OSH], DT_W, kind="ExternalOutput")

    with tile.TileContext(nc) as tc:
        with (
            tc.tile_pool(name="sb", bufs=1) as sb,
            tc.tile_pool(name="ps", bufs=1, space="PSUM") as ps,
        ):
            # psum: 5 banks (dummies share trep's bank)
            qa_ps = ps.tile([BATCH, 512], f32, name="qa", tag="qa")
            qb_ps = ps.tile([BATCH, 512], f32, name="qb", tag="qb")
            trep_ps = ps.tile([128, 512], f32, name="trep", tag="trep")
            lora_ps = ps.tile([128, 512], f32, name="lora", tag="lora")
            base_ps = ps.tile([BATCH, OSH], f32, name="base", tag="base")

            # ---- PE warm-up: dummy matmuls on memset junk (no DMA deps).
            junk_w = sb.tile([128, BATCH], DT_F8)
            nc.vector.memset(junk_w[:], 0.0)
            junk_r = sb.tile([128, 512], DT_F8)
            nc.gpsimd.memset(junk_r[:], 0.0)
            for i in range(N_DUMMY):
                nc.tensor.matmul(trep_ps[:BATCH, :], junk_w[:], junk_r[:],
                                 start=True, stop=True)

            # ---- DMA ring1 (sync): s1+wa x2, wb.  ring2 (scalar): s2, wt x2.
            wa_sb = sb.tile([128, S1C + IC * KL], DT_F8)
            half = S1C + 8 * KL
            nc.sync.dma_start(wa_sb[:, :half], wa[:, :half])
            nc.sync.dma_start(wa_sb[:, half:], wa[:, half:])
            wb_sb = sb.tile([128, 4 * OUT_F], DT_F8)
            nc.sync.dma_start(wb_sb[:], wb[:])

            s2_sb = sb.tile([128, S2X + S2H + S2HD], DT_WT)
            nc.scalar.dma_start(s2_sb[:], s2[:])
            wt_sb = sb.tile([128, IC * OSH], DT_WT)
            nc.scalar.dma_start(wt_sb[:, :8 * OSH], wt[:, :8 * OSH])
            nc.scalar.dma_start(wt_sb[:, 8 * OSH:], wt[:, 8 * OSH:])

            # ---- Q phase (DoubleRow), two psum groups so t_a overlaps
            # the second wa chunk's DMA.
            s1_ap = wa_sb[:, :S1C]

            def q_mm(ic2, dst, start, stop):
                nc.tensor.matmul(
                    dst,
                    s1_ap[:, ic2 * 32:(ic2 + 1) * 32]
                    .rearrange("p (k b) -> p k b", k=2),
                    wa_sb[:, S1C + ic2 * 2 * KL:S1C + (ic2 + 1) * 2 * KL]
                    .rearrange("p (k n) -> p k n", k=2),
                    start=start, stop=stop, perf_mode=DR)

            for ic2 in range(4):
                q_mm(ic2, qa_ps[:], ic2 == 0, ic2 == 3)
            for ic2 in range(4, 8):
                q_mm(ic2, qb_ps[:], ic2 == 4, ic2 == 7)

            # ---- base phase first half: fills the PE idle window while
            # the t-chain runs on DVE.
            def base_mm(ic):
                nc.tensor.matmul(
                    base_ps[:],
                    s2_sb[:, ic * BATCH:(ic + 1) * BATCH],
                    wt_sb[:, ic * OSH:(ic + 1) * OSH],
                    start=(ic == 0), stop=(ic == IC - 1),
                )
            for ic in range(8):
                base_mm(ic)

            # ---- t[b,r] = sum_d (qa+qb)[b,(r,d)] * h[b,d]   (DVE)
            h_ap = s2_sb[:BATCH, S2X + S2H:S2X + S2H + HDIM]
            dm_ap = s2_sb[:BATCH, S2X + S2H + HDIM:]
            ta_sb = sb.tile([BATCH, RL], f32)
            tb_sb = sb.tile([BATCH, RL], f32)
            t_sb = sb.tile([BATCH, RL], f32)
            tt_scr = sb.tile([BATCH, HDIM], f32)
            for r in range(RL):
                nc.vector.scalar_tensor_tensor(
                    out=tt_scr[:], in0=qa_ps[:, r * HDIM:(r + 1) * HDIM],
                    scalar=1.0, in1=h_ap,
                    op0=mybir.AluOpType.mult, op1=mybir.AluOpType.mult,
                    accum_out=ta_sb[:, r:r + 1])
            for r in range(RL):
                nc.vector.scalar_tensor_tensor(
                    out=tt_scr[:], in0=qb_ps[:, r * HDIM:(r + 1) * HDIM],
                    scalar=1.0, in1=h_ap,
                    op0=mybir.AluOpType.mult, op1=mybir.AluOpType.mult,
                    accum_out=tb_sb[:, r:r + 1])
            nc.vector.tensor_add(t_sb[:], ta_sb[:], tb_sb[:])

            # ---- replicate t across partitions: trep = ones16.T @ (dm * t)
            ones16 = sb.tile([BATCH, 128], DT_W)
            nc.vector.memset(ones16[:], 1.0)
            rhs_t = sb.tile([BATCH, BATCH * RL], DT_W)
            nc.vector.tensor_mul(
                rhs_t[:].rearrange("k (b r) -> k b r", r=RL),
                dm_ap.rearrange("k (b r) -> k b r", r=RL),
                t_sb[:].unsqueeze(1).broadcast_to((BATCH, BATCH, RL)))
            nc.tensor.matmul(trep_ps[:, :BATCH * RL], ones16[:], rhs_t[:],
                             start=True, stop=True)
            # gT[(dh,p), (r, dh', b)] = h[b, dh'*128+p] * t[b, r]
            ht_ap = s2_sb[:, S2X:S2X + S2H]
            g_sb = sb.tile([128, RL * 2 * BATCH], DT_F8)
            nc.vector.tensor_mul(
                g_sb[:].rearrange("p (r k b) -> p r k b", r=RL, k=2),
                ht_ap.rearrange("p (k b) -> p k b", k=2)
                .unsqueeze(1).broadcast_to((128, RL, 2, BATCH)),
                trep_ps[:, :BATCH * RL].rearrange("p (b r) -> p r b", r=RL)
                .unsqueeze(2).broadcast_to((128, RL, 2, BATCH)))

            # ---- base phase second half
            for ic in range(8, IC):
                base_mm(ic)

            # ---- lora: 16 plain fp8 matmuls, col-group tiled.  Bank nn
            # lives at partitions [32nn, 32nn+16) of ONE psum bank; the
            # four banks execute concurrently in the PE's col groups.
            # wb layout is kc-major: wb_sb[:, kc*2048 + nn*512 : +512].
            for kc in range(4):
                for nn in range(4):
                    nc.tensor.matmul(
                        lora_ps[32 * nn:32 * nn + BATCH, :],
                        g_sb[:, kc * BATCH:(kc + 1) * BATCH],
                        wb_sb[:, kc * OUT_F + nn * 512:
                              kc * OUT_F + (nn + 1) * 512],
                        start=(kc == 0), stop=(kc == 3),
                        tile_position=(0, 32 * nn),
                    )

            # ---- copies + outputs
            outm_sb = sb.tile([128, 512], DT_W)
            outb_sb = sb.tile([BATCH, OSH], DT_W)
            nc.vector.tensor_scalar_mul(outb_sb[:], base_ps[:], 1.0)
            nc.sync.dma_start(out_b[:], outb_sb[:])
            nc.vector.tensor_scalar_mul(outm_sb[:], lora_ps[:], LORA_DESCALE)
            nc.sync.dma_start(out_m[:], outm_sb[:])

    nc.compile()
    return nc


_NC_CACHE = None


def _get_nc():
    global _NC_CACHE
    if _NC_CACHE is None:
        _NC_CACHE = _build()
    return _NC_CACHE


def _interleave(a, p=128):
    """[C*p, F] -> [p, C*F]: the SBUF layout used on device."""
    c = a.shape[0] // p
    return np.ascontiguousarray(
        a.reshape(c, p, a.shape[1]).transpose(1, 0, 2).reshape(p, -1))


def _prep(x, HN_ids, layer_id, weight, bias, emb_id, emb_layer, W_A, W_B):
    """Host-side layout prep + sharding. Returns in_maps for 8 cores."""
    f32 = np.float32
    x = np.asarray(x, f32)
    weight = np.asarray(weight, f32)
    emb_id = np.asarray(emb_id, f32)
    emb_layer = np.asarray(emb_layer, f32)
    W_A = np.asarray(W_A, f32)
    W_B = np.asarray(W_B, f32)
    ids = np.asarray(HN_ids).astype(np.int64)
    lid = int(np.asarray(layer_id))

    h = emb_id[ids] + emb_layer[lid]                      # [B, HDIM]

    np_wt, np_f8 = _np_dt(DT_WT), _np_dt(DT_F8)

    s1 = _interleave(np.ascontiguousarray(x.T)).astype(np_f8)
    wa3 = W_A.reshape(HDIM, R, IN_F)
    wa_all = np.ascontiguousarray(
        wa3.transpose(2, 1, 0) * WA_SCALE).astype(np_f8)
    wb3 = W_B.reshape(HDIM, OUT_F, R)
    wb_all = np.ascontiguousarray(
        wb3.transpose(2, 0, 1) * WB_SCALE).astype(np_f8)
    ht = _interleave(np.ascontiguousarray(h.T)).astype(np_wt)
    xt_il = _interleave(np.ascontiguousarray(x.T)).astype(np_wt)
    dm = np.zeros((BATCH, BATCH, RL), f32)
    dm[np.arange(BATCH), np.arange(BATCH), :] = G_SCALE
    hd = np.zeros((128, S2HD), f32)
    hd[:BATCH, :HDIM] = h / WA_SCALE
    hd[:BATCH, HDIM:] = dm.reshape(BATCH, BATCH * RL)
    s2 = np.concatenate([xt_il, ht, hd.astype(np_wt)], axis=1)
    wt_full = np.ascontiguousarray(weight.T)              # [in, out]

    in_maps = []
    for c in range(N_CORES):
        sl = slice(c * OSH, (c + 1) * OSH)
        rsl = slice(c * RL, (c + 1) * RL)
        wb_c = np.roll(wb_all[rsl], -c * OSH, axis=2).reshape(KL, OUT_F)
        wb_il = np.concatenate(
            [wb_c[kc * 128:(kc + 1) * 128, :] for kc in range(4)], axis=1)
        wa_c = _interleave(np.ascontiguousarray(
            wa_all[:, rsl, :]).reshape(IN_F, KL))
        in_maps.append({
            "wa": np.concatenate([s1, wa_c], axis=1),
            "wb": np.ascontiguousarray(wb_il),
            "s2": s2,
            "wt": _interleave(
                np.ascontiguousarray(wt_full[:, sl]).astype(np_wt)),
        })
    return in_maps


def kernel(**inputs):
    nc = _get_nc()
    in_maps = _prep(**inputs)
    res = run_bass_kernel_spmd(nc, in_maps, core_ids=list(range(N_CORES)))
    bias = np.asarray(inputs["bias"], np.float32)
    out = np.zeros((BATCH, OUT_F), np.float32)
    for c in range(N_CORES):
        m = np.asarray(res.results[c]["out_m"], np.float32)  # [128, 512]
        rot = np.concatenate(
            [m[32 * nn:32 * nn + BATCH, :] for nn in range(4)], axis=1)
        rot[:, :OSH] += np.asarray(res.results[c]["out_b"], np.float32)
        out += np.roll(rot, c * OSH, axis=1)
    return (out + bias[None, :]).astype(np.float32)


def run_traced(inputs, n=3):
    """Timing helper for test.py: returns (exec_times_ns, last_results)."""
    nc = _get_nc()
    in_maps = _prep(**inputs)
    times = []
    res = None
    for _ in range(n):
        res = run_bass_kernel_spmd(nc, in_maps, core_ids=list(range(N_CORES)),
                                   trace=True)
        times.append(res.exec_time_ns)
    return times, res


# revision 10
# speedup vs baseline: 1.0945x; 1.0941x over previous
"""Trainium2 Bass kernel for nn_AdaptedLinear (hypernetwork-adapted linear).

Math (per sample b):
  h = emb_id[HN_ids[b]] + emb_layer[layer_id]                 # [256]
  A = (h @ W_A).reshape(R, IN)    t = A @ x_b                 # [16]
  B = (h @ W_B).reshape(OUT, R)
  out_b = weight @ x_b + B @ t + bias                         # never materialize delta

Distribution across 8 NeuronCores -- no collectives:
  - LoRA path sharded by rank: core c owns ranks {2c, 2c+1}; each core
    emits a partial lora [batch, out_full] in a per-core-rotated layout
    (wb columns rolled by -c*OSH on host); the base shard ships as a
    separate small output.  Host un-rotates, sums, adds base and bias.
  - weight (base path) sharded by output dim (256 cols/core).

v7 schedule:
  - Bulk streams on the two HWDGE rings (sync=SP, scalar=ACT), <=5
    dispatches per ring (deeper queues stall the dispatcher for ~3-5us).
    ring1 (sync):  [s1|wa ics0-7], [wa ics8-15], wb, out_main, out_base
    ring2 (scalar): [s2|ht|h|dmask], wt ics0-7, wt ics8-15
  - Q (DoubleRow fp8) accumulates in two psum groups; the first half of
    the t-reduction runs during the second wa chunk's DMA wait.
  - lora: 16 plain fp8 matmuls col-GROUP-TILED across the PE array --
    bank nn writes lora_all[32nn:32nn+16, :512] with
    tile_position=(0,32nn), so the four banks execute concurrently in
    the four 32-column sub-array groups of ONE psum bank.  The whole
    lora tail after the wb sem is ~1.2us.
  - result copy is ONE [128,512] DVE op (full 128 partitions) and the
    main output DMA is [128,512] bf16 = 128KB at full partition rate.
  - base matmuls (own bank) fill PE idle windows (wt in 2 chunks);
    base ships as its own [16,256] bf16 output, added on host.
  - 8 dummy matmuls at t=0 warm the PE HAM clock gate.
"""

import sys

sys.path.insert(0, "/opt/trn_rl_repo")

import numpy as np

import concourse.bass as bass
import concourse.bacc as bacc
import concourse.tile as tile
import concourse.mybir as mybir
from concourse.bass_utils import run_bass_kernel_spmd

IN_F, OUT_F, R = 2048, 2048, 16
HDIM = 256
BATCH = 16
N_CORES = 8
OSH = OUT_F // N_CORES     # 256 base-output cols per core
RL = R // N_CORES          # 2 local ranks per core
KL = RL * HDIM             # 512 local lora contraction rows

DT_W = mybir.dt.bfloat16
DT_F8 = mybir.dt.float8e4
WB_SCALE = 256.0
G_SCALE = 64.0
WA_SCALE = 256.0
LORA_DESCALE = 1.0 / (WB_SCALE * G_SCALE)
DT_WT = mybir.dt.bfloat16

IC = IN_F // 128           # 16 i-chunks
N_DUMMY = 8                # PE warm-up matmuls (N=512 each)
DR = mybir.MatmulPerfMode.DoubleRow

S1C = IC * BATCH           # 256 cols of s1 inside the wa tensor
S2X = IC * BATCH           # x cols in s2
S2H = 2 * BATCH            # ht cols in s2
S2HD = HDIM + BATCH * RL   # h + dmask cols (rows 0:16 only)


def _np_dt(dt):
    return np.dtype(mybir.dt.np(dt))


def _build():
    nc = bacc.Bacc("TRN2", target_bir_lowering=False, debug=False,
                   num_devices=N_CORES)
    f32 = mybir.dt.float32

    wa = nc.dram_tensor("wa", [128, S1C + IC * KL], DT_F8,
                        kind="ExternalInput")
    wb = nc.dram_tensor("wb", [128, 4 * OUT_F], DT_F8, kind="ExternalInput")
    s2 = nc.dram_tensor("s2", [128, S2X + S2H + S2HD], DT_WT,
                        kind="ExternalInput")
    wt = nc.dram_tensor("wt", [128, IC * OSH], DT_WT, kind="ExternalInput")
    out_m = nc.dram_tensor("out_m", [128, 512], DT_W, kind="ExternalOutput")
    out_b = nc.dram_tensor("out_b", [BATCH, OSH], DT_W, kind="ExternalOutput")

    with tile.TileContext(nc) as tc:
        with (
            tc.tile_pool(name="sb", bufs=1) as sb,
            tc.tile_pool(name="ps", bufs=1, space="PSUM") as ps,
        ):
            # psum: 5 banks (dummies share trep's bank)
            qa_ps = ps.tile([BATCH, 512], f32, name="qa", tag="qa")
            qb_ps = ps.tile([BATCH, 512], f32, name="qb", tag="qb")
            trep_ps = ps.tile([128, 512], f32, name="trep", tag="trep")
            lora_ps = ps.tile([128, 512], f32, name="lora", tag="lora")
            base_ps = ps.tile([BATCH, OSH], f32, name="base", tag="base")

            # ---- PE warm-up: dummy matmuls on memset junk (no DMA deps).
            junk_w = sb.tile([128, BATCH], DT_F8)
            nc.vector.memset(junk_w[:], 0.0)
            junk_r = sb.tile([128, 512], DT_F8)
            nc.gpsimd.memset(junk_r[:], 0.0)
            for i in range(N_DUMMY):
                nc.tensor.matmul(trep_ps[:BATCH, :], junk_w[:], junk_r[:],
                                 start=True, stop=True)

            # ---- DMA ring1 (sync): s1+wa x2, wb.  ring2 (scalar): s2, wt x2.
            wa_sb = sb.tile([128, S1C + IC * KL], DT_F8)
            half = S1C + 8 * KL
            nc.sync.dma_start(wa_sb[:, :half], wa[:, :half])
            nc.sync.dma_start(wa_sb[:, half:], wa[:, half:])
            wb_sb = sb.tile([128, 4 * OUT_F], DT_F8)
            nc.sync.dma_start(wb_sb[:, :2 * OUT_F], wb[:, :2 * OUT_F])
            nc.sync.dma_start(wb_sb[:, 2 * OUT_F:3 * OUT_F],
                              wb[:, 2 * OUT_F:3 * OUT_F])
            nc.sync.dma_start(wb_sb[:, 3 * OUT_F:], wb[:, 3 * OUT_F:])

            s2_sb = sb.tile([128, S2X + S2H + S2HD], DT_WT)
            nc.scalar.dma_start(s2_sb[:], s2[:])
            wt_sb = sb.tile([128, IC * OSH], DT_WT)
            nc.scalar.dma_start(wt_sb[:, :8 * OSH], wt[:, :8 * OSH])
            nc.scalar.dma_start(wt_sb[:, 8 * OSH:], wt[:, 8 * OSH:])

            # ---- Q phase (DoubleRow), two psum groups so t_a overlaps
            # the second wa chunk's DMA.
            s1_ap = wa_sb[:, :S1C]

            def q_mm(ic2, dst, start, stop):
                nc.tensor.matmul(
                    dst,
                    s1_ap[:, ic2 * 32:(ic2 + 1) * 32]
                    .rearrange("p (k b) -> p k b", k=2),
                    wa_sb[:, S1C + ic2 * 2 * KL:S1C + (ic2 + 1) * 2 * KL]
                    .rearrange("p (k n) -> p k n", k=2),
                    start=start, stop=stop, perf_mode=DR)

            def base_mm(ic):
                nc.tensor.matmul(
                    base_ps[:],
                    s2_sb[:, ic * BATCH:(ic + 1) * BATCH],
                    wt_sb[:, ic * OSH:(ic + 1) * OSH],
                    start=(ic == 0), stop=(ic == IC - 1),
                )

            for ic2 in range(4):
                q_mm(ic2, qa_ps[:], ic2 == 0, ic2 == 3)
            for ic in range(8):
                base_mm(ic)
            for ic2 in range(4, 8):
                q_mm(ic2, qb_ps[:], ic2 == 4, ic2 == 7)

            # ---- t[b,r] = sum_d (qa+qb)[b,(r,d)] * h[b,d]   (DVE)
            h_ap = s2_sb[:BATCH, S2X + S2H:S2X + S2H + HDIM]
            dm_ap = s2_sb[:BATCH, S2X + S2H + HDIM:]
            ta_sb = sb.tile([BATCH, RL], f32)
            tb_sb = sb.tile([BATCH, RL], f32)
            t_sb = sb.tile([BATCH, RL], f32)
            tt_scr = sb.tile([BATCH, HDIM], f32)
            for r in range(RL):
                nc.vector.scalar_tensor_tensor(
                    out=tt_scr[:], in0=qa_ps[:, r * HDIM:(r + 1) * HDIM],
                    scalar=1.0, in1=h_ap,
                    op0=mybir.AluOpType.mult, op1=mybir.AluOpType.mult,
                    accum_out=ta_sb[:, r:r + 1])
            for r in range(RL):
                nc.vector.scalar_tensor_tensor(
                    out=tt_scr[:], in0=qb_ps[:, r * HDIM:(r + 1) * HDIM],
                    scalar=1.0, in1=h_ap,
                    op0=mybir.AluOpType.mult, op1=mybir.AluOpType.mult,
                    accum_out=tb_sb[:, r:r + 1])
            nc.vector.tensor_add(t_sb[:], ta_sb[:], tb_sb[:])

            # ---- replicate t across partitions: trep = ones16.T @ (dm * t)
            ones16 = sb.tile([BATCH, 128], DT_W)
            nc.vector.memset(ones16[:], 1.0)
            rhs_t = sb.tile([BATCH, BATCH * RL], DT_W)
            nc.vector.tensor_mul(
                rhs_t[:].rearrange("k (b r) -> k b r", r=RL),
                dm_ap.rearrange("k (b r) -> k b r", r=RL),
                t_sb[:].unsqueeze(1).broadcast_to((BATCH, BATCH, RL)))
            nc.tensor.matmul(trep_ps[:, :BATCH * RL], ones16[:], rhs_t[:],
                             start=True, stop=True)
            # gT[(dh,p), (r, dh', b)] = h[b, dh'*128+p] * t[b, r]
            ht_ap = s2_sb[:, S2X:S2X + S2H]
            g_sb = sb.tile([128, RL * 2 * BATCH], DT_F8)
            nc.vector.tensor_mul(
                g_sb[:].rearrange("p (r k b) -> p r k b", r=RL, k=2),
                ht_ap.rearrange("p (k b) -> p k b", k=2)
                .unsqueeze(1).broadcast_to((128, RL, 2, BATCH)),
                trep_ps[:, :BATCH * RL].rearrange("p (b r) -> p r b", r=RL)
                .unsqueeze(2).broadcast_to((128, RL, 2, BATCH)))

            # ---- base phase second half
            for ic in range(8, IC):
                base_mm(ic)

            # ---- lora: 16 plain fp8 matmuls, col-group tiled.  Bank nn
            # lives at partitions [32nn, 32nn+16) of ONE psum bank; the
            # four banks execute concurrently in the PE's col groups.
            # wb layout is kc-major: wb_sb[:, kc*2048 + nn*512 : +512].
            for kc in range(4):
                for nn in range(4):
                    nc.tensor.matmul(
                        lora_ps[32 * nn:32 * nn + BATCH, :],
                        g_sb[:, kc * BATCH:(kc + 1) * BATCH],
                        wb_sb[:, kc * OUT_F + nn * 512:
                              kc * OUT_F + (nn + 1) * 512],
                        start=(kc == 0), stop=(kc == 3),
                        tile_position=(0, 32 * nn),
                    )

            # ---- copies + outputs
            outm_sb = sb.tile([128, 512], DT_W)
            outb_sb = sb.tile([BATCH, OSH], DT_W)
            nc.vector.tensor_scalar_mul(outb_sb[:], base_ps[:], 1.0)
            nc.sync.dma_start(out_b[:], outb_sb[:])
            nc.vector.tensor_scalar_mul(outm_sb[:], lora_ps[:], LORA_DESCALE)
            nc.sync.dma_start(out_m[:], outm_sb[:])

    nc.compile()
    return nc


_NC_CACHE = None


def _get_nc():
    global _NC_CACHE
    if _NC_CACHE is None:
        _NC_CACHE = _build()
    return _NC_CACHE


def _interleave(a, p=128):
    """[C*p, F] -> [p, C*F]: the SBUF layout used on device."""
    c = a.shape[0] // p
    return np.ascontiguousarray(
        a.reshape(c, p, a.shape[1]).transpose(1, 0, 2).reshape(p, -1))


def _prep(x, HN_ids, layer_id, weight, bias, emb_id, emb_layer, W_A, W_B):
    """Host-side layout prep + sharding. Returns in_maps for 8 cores."""
    f32 = np.float32
    x = np.asarray(x, f32)
    weight = np.asarray(weight, f32)
    emb_id = np.asarray(emb_id, f32)
    emb_layer = np.asarray(emb_layer, f32)
    W_A = np.asarray(W_A, f32)
    W_B = np.asarray(W_B, f32)
    ids = np.asarray(HN_ids).astype(np.int64)
    lid = int(np.asarray(layer_id))

    h = emb_id[ids] + emb_layer[lid]                      # [B, HDIM]

    np_wt, np_f8 = _np_dt(DT_WT), _np_dt(DT_F8)

    s1 = _interleave(np.ascontiguousarray(x.T)).astype(np_f8)
    wa3 = W_A.reshape(HDIM, R, IN_F)
    wa_all = np.ascontiguousarray(
        wa3.transpose(2, 1, 0) * WA_SCALE).astype(np_f8)
    wb3 = W_B.reshape(HDIM, OUT_F, R)
    wb_all = np.ascontiguousarray(
        wb3.transpose(2, 0, 1) * WB_SCALE).astype(np_f8)
    ht = _interleave(np.ascontiguousarray(h.T)).astype(np_wt)
    xt_il = _interleave(np.ascontiguousarray(x.T)).astype(np_wt)
    dm = np.zeros((BATCH, BATCH, RL), f32)
    dm[np.arange(BATCH), np.arange(BATCH), :] = G_SCALE
    hd = np.zeros((128, S2HD), f32)
    hd[:BATCH, :HDIM] = h / WA_SCALE
    hd[:BATCH, HDIM:] = dm.reshape(BATCH, BATCH * RL)
    s2 = np.concatenate([xt_il, ht, hd.astype(np_wt)], axis=1)
    wt_full = np.ascontiguousarray(weight.T)              # [in, out]

    in_maps = []
    for c in range(N_CORES):
        sl = slice(c * OSH, (c + 1) * OSH)
        rsl = slice(c * RL, (c + 1) * RL)
        wb_c = np.roll(wb_all[rsl], -c * OSH, axis=2).reshape(KL, OUT_F)
        wb_il = np.concatenate(
            [wb_c[kc * 128:(kc + 1) * 128, :] for kc in range(4)], axis=1)
        wa_c = _interleave(np.ascontiguousarray(
            wa_all[:, rsl, :]).reshape(IN_F, KL))
        in_maps.append({
            "wa": np.concatenate([s1, wa_c], axis=1),
            "wb": np.ascontiguousarray(wb_il),
            "s2": s2,
            "wt": _interleave(
                np.ascontiguousarray(wt_full[:, sl]).astype(np_wt)),
        })
    return in_maps


def kernel(**inputs):
    nc = _get_nc()
    in_maps = _prep(**inputs)
    res = run_bass_kernel_spmd(nc, in_maps, core_ids=list(range(N_CORES)))
    bias = np.asarray(inputs["bias"], np.float32)
    out = np.zeros((BATCH, OUT_F), np.float32)
    for c in range(N_CORES):
        m = np.asarray(res.results[c]["out_m"], np.float32)  # [128, 512]
        rot = np.concatenate(
            [m[32 * nn:32 * nn + BATCH, :] for nn in range(4)], axis=1)
        rot[:, :OSH] += np.asarray(res.results[c]["out_b"], np.float32)
        out += np.roll(rot, c * OSH, axis=1)
    return (out + bias[None, :]).astype(np.float32)


def run_traced(inputs, n=3):
    """Timing helper for test.py: returns (exec_times_ns, last_results)."""
    nc = _get_nc()
    in_maps = _prep(**inputs)
    times = []
    res = None
    for _ in range(n):
        res = run_bass_kernel_spmd(nc, in_maps, core_ids=list(range(N_CORES)),
                                   trace=True)
        times.append(res.exec_time_ns)
    return times, res
